# revision 1
# baseline (speedup 1.0000x reference)
"""Trainium2 Bass kernel for structured-sparse matmul.

Computes: out[b,s,o] = sum_k x[b,s,sparse_idx[k]] * sparse_values[o,k]
  x: [4, 2048, 4096] f32, sparse_values: [4096, 1024] f32,
  sparse_idx: [1024] int64 (sorted, unique) -> out [4, 2048, 4096] f32

Strategy (8 NeuronCores, data-parallel over rows m = b*s):
  Per core (M=1024 rows):
    Phase A (on device): PE-transpose x tiles (regular f32r matmul against
      an identity), then gather n->k via matmul with one-hot selection
      blocks G (built on the host from sparse_idx, which is compile-time
      metadata): x_gT[k, m].
    Phase B: GEMM out[m, o] = x_gT.T @ W^T[k, o] with float32r
      (full-rate fp22 multiplies, fp32 accumulate).
  Host only slices x, transposes sparse_values (weight layout prep),
  and expands sparse_idx into the tiny selection blocks.

Measured (neuron-profile, core 0): 208.6 us HW exec with x chunks alternated
across the sync/scalar HWDGE rings (vs 213-225 us, median ~218, single-ring;
alternation cuts head-of-line blocking in the in-order x stream). PE ~185 us
busy; the 512 GEMM matmuls run at ~222 ns per [128x128x512] f32r MM
(stream-rate, 91% PE occupancy). Remaining span = ~10 us startup dead zone +
~12 us Tile drain tail + x-delivery jitter.
Relative error vs the fp32 reference: 1.47e-4 (fp22 multiply truncation).
"""

import sys

if "/opt/trn_rl_repo" not in sys.path:
    sys.path.insert(0, "/opt/trn_rl_repo")

import numpy as np

B, S, N_IN = 4, 2048, 4096
N_OUT, N_SPARSE = 4096, 1024
N_CORES = 8
M_TOT = B * S            # 8192
M = M_TOT // N_CORES     # 1024 rows per core
P = 128
NKT = N_SPARSE // P      # 8 k-tiles
NNB = N_IN // P          # 32 n-blocks
N_MT = M // P            # 8 m-tiles per core
MB = 512                 # m-batch for transpose/gather staging
N_BATCH = M // MB        # 2
MSUB = MB // P           # 4 m-subtiles per batch
NQ = 4                   # x streamed in quarter-width column chunks
NQW = N_IN // NQ         # 1024 columns per chunk
O_TILE = 512
NOS = N_OUT // O_TILE    # 8 o-slices

_cache: dict = {}


def _build_gather_blocks(idx: np.ndarray):
    """Expand sparse_idx into one-hot selection blocks.

    For k-tile kt and n-block b, G[n, krel] = 1 iff idx[kt*128+krel] == b*128+n.
    Returns (g_all [NB,128,128] f32, blocks_per_kt: list of lists of (bi, b)).
    """
    mats = []
    blocks_per_kt = []
    for kt in range(NKT):
        ks = idx[kt * P:(kt + 1) * P]
        bs = sorted(set(int(k) // P for k in ks))
        entries = []
        for b in bs:
            mat = np.zeros((P, P), dtype=np.float32)
            for krel, k in enumerate(ks):
                if int(k) // P == b:
                    mat[int(k) % P, krel] = 1.0
            entries.append((len(mats), b))
            mats.append(mat)
        blocks_per_kt.append(entries)
    return np.stack(mats), blocks_per_kt


def _build_nc(blocks_per_kt, nb_total):
    import concourse.mybir as mybir
    import concourse.tile as tile
    from concourse import bacc

    F32R = mybir.dt.float32r
    F32 = mybir.dt.float32

    nc = bacc.Bacc("TRN2", target_bir_lowering=False, debug=False)
    x = nc.dram_tensor("x", [M, N_IN], F32R, kind="ExternalInput")
    wt = nc.dram_tensor("wt", [NOS, P, NKT, O_TILE], F32R, kind="ExternalInput")
    g = nc.dram_tensor("g", [P, nb_total, P], F32R, kind="ExternalInput")
    ident = nc.dram_tensor("ident", [P, P], F32R, kind="ExternalInput")
    out = nc.dram_tensor("out", [M, N_OUT], F32, kind="ExternalOutput")

    with tile.TileContext(nc) as tc:
        with (
            tc.tile_pool(name="const", bufs=1) as const_pool,
            tc.tile_pool(name="gpool", bufs=1) as g_pool,
            tc.tile_pool(name="xgpool", bufs=1) as xg_pool,
            tc.tile_pool(name="xin", bufs=2) as x_pool,
            tc.tile_pool(name="xtpool", bufs=1) as xt_pool,
            tc.tile_pool(name="wpool", bufs=2) as wt_pool,
            tc.tile_pool(name="opool", bufs=4) as o_pool,
            tc.tile_pool(name="ps_t", bufs=3, space="PSUM") as pst,
            tc.tile_pool(name="ps_g", bufs=2, space="PSUM") as psg,
            tc.tile_pool(name="ps_b", bufs=3, space="PSUM") as psb,
        ):
            ident_sb = const_pool.tile([P, P], F32R)
            nc.sync.dma_start(ident_sb[:], ident[:])
            # x_gT resident: [k-part, kt, m]
            xg_sb = xg_pool.tile([P, NKT, M], F32R)

            # PE warm-up: the HAM clock gate keeps the PE at 1.2 GHz until it
            # sees ~3.4us of sustained activity. Burn dummy matmuls while the
            # first x tile is still in flight so the real work runs at 2.4.
            for w in range(22):
                wps = psb.tile([P, O_TILE], F32, tag="psb", name=f"warm{w}")
                nc.tensor.matmul(
                    wps[:, :P], ident_sb[:], ident_sb[:], start=True, stop=True
                )

            # wt slices are prefetched on the scalar (ACT) HWDGE ring so they
            # don't queue behind the x loads on the sync ring.
            wt_tiles = {}

            def ensure_wt(s):
                if s >= NOS or s in wt_tiles:
                    return
                t = wt_pool.tile([P, NKT, O_TILE], F32R, tag="wt", name=f"wt{s}")
                nc.scalar.dma_start(t[:, :NKT // 2, :], wt[s, :, :NKT // 2, :])
                nc.scalar.dma_start(t[:, NKT // 2:, :], wt[s, :, NKT // 2:, :])
                wt_tiles[s] = t

            def emit_b(s, t_range, prefetch_at=None, prefetch_s=None):
                wt_sb = wt_tiles[s]
                for t in t_range:
                    ps = psb.tile([P, O_TILE], F32, tag="psb",
                                  name=f"psb{s}_{t}")
                    for kt in range(NKT):
                        nc.tensor.matmul(
                            ps[:],
                            xg_sb[:, kt, t * P:(t + 1) * P],
                            wt_sb[:, kt, :],
                            start=(kt == 0),
                            stop=(kt == NKT - 1),
                        )
                    o_sb = o_pool.tile([P, O_TILE], F32, tag="ob",
                                       name=f"ob{s}_{t}")
                    # DVE is idle during the GEMM phase; keep ACT free for
                    # the wt prefetch DMAs.
                    nc.vector.tensor_copy(o_sb[:], ps[:])
                    nc.sync.dma_start(
                        out[t * P:(t + 1) * P, s * O_TILE:(s + 1) * O_TILE],
                        o_sb[:],
                    )
                    if prefetch_at is not None and t == prefetch_at:
                        ensure_wt(prefetch_s)

            # ---- Phase A: transpose + gather ----
            g_sb = None
            for batch in range(N_BATCH):
                m0 = batch * MB
                xt_sb = xt_pool.tile([P, NNB, MB], F32R, tag="xt")

                def emit_gather(kt):
                    entries = blocks_per_kt[kt]
                    ps = psg.tile([P, MB], F32, tag="psg", name=f"psg{batch}_{kt}")
                    for i, (bi, b) in enumerate(entries):
                        nc.tensor.matmul(
                            ps[:],
                            g_sb[:, bi, :],
                            xt_sb[:, b, :],
                            start=(i == 0),
                            stop=(i == len(entries) - 1),
                        )
                    nc.scalar.copy(xg_sb[:, kt, m0:m0 + MB], ps[:])

                gathered = set()
                NBQ = NNB // NQ  # n-blocks per chunk (8)
                for q in range(NQ):
                    # x streamed as [128 part(m), MSUB, 1024 cols] quarter
                    # tiles, loaded by per-j 512KB DMAs so transposes start
                    # as soon as one m-subtile lands.
                    x_sb = x_pool.tile([P, MSUB, NQW], F32R, tag="xin")
                    for j in range(MSUB):
                        eng = nc.sync if j % 2 == 0 else nc.scalar
                        eng.dma_start(
                            x_sb[:, j, :],
                            x[m0 + j * P:m0 + (j + 1) * P,
                              q * NQW:(q + 1) * NQW],
                        )
                    if batch == 0 and q == 1:
                        # g lands behind q0/q1's scalar-ring chunks, just
                        # ahead of the first gathers (~25us in).
                        g_sb = g_pool.tile([P, nb_total, P], F32R)
                        nc.scalar.dma_start(g_sb[:], g[:])
                        ensure_wt(0)
                        ensure_wt(1)
                    for j in range(MSUB):
                        for half in range(2):
                            ps = pst.tile([P, NBQ // 2, P], F32, tag="pst")
                            for nbq in range(NBQ // 2):
                                c = half * (NBQ // 2) + nbq
                                nc.tensor.matmul(
                                    ps[:, nbq, :],
                                    x_sb[:, j, c * P:(c + 1) * P],
                                    ident_sb[:],
                                    start=True,
                                    stop=True,
                                )
                            nb0 = q * NBQ + half * (NBQ // 2)
                            dst = xt_sb[:, nb0:nb0 + NBQ // 2,
                                        j * P:(j + 1) * P]
                            # Alternate eviction engine so neither DVE nor
                            # ACT gates PSUM recycling.
                            if (j + half) % 2 == 0:
                                nc.vector.tensor_copy(dst, ps[:])
                            else:
                                nc.scalar.copy(dst, ps[:])
                    # Emit every gather whose source n-blocks are now all
                    # transposed — keeps the PE stream dense and spreads the
                    # gather work across the batch instead of bunching it.
                    nb_done = (q + 1) * (NNB // NQ)
                    for kt in range(NKT):
                        if kt in gathered or g_sb is None:
                            continue
                        if all(b < nb_done for _, b in blocks_per_kt[kt]):
                            emit_gather(kt)
                            gathered.add(kt)
                for kt in range(NKT):
                    if kt not in gathered:
                        emit_gather(kt)

            # ---- Phase B: main GEMM ----
            for s in range(NOS):
                ensure_wt(s)
                emit_b(s, range(N_MT), prefetch_at=2, prefetch_s=s + 2)
    nc.compile()
    return nc


def _get_compiled(idx: np.ndarray):
    key = idx.tobytes()
    if key not in _cache:
        g_all, blocks_per_kt = _build_gather_blocks(idx)
        nc = _build_nc(blocks_per_kt, g_all.shape[0])
        _cache[key] = (nc, g_all)
    return _cache[key]


def _run(inputs, trace=False, trace_kwargs=None):
    from concourse.bass_utils import run_bass_kernel_spmd

    x = np.ascontiguousarray(np.asarray(inputs["x"], dtype=np.float32))
    sv = np.asarray(inputs["sparse_values"], dtype=np.float32)
    idx = np.asarray(inputs["sparse_idx"]).astype(np.int64)

    nc, g_all = _get_compiled(idx)

    x2 = x.reshape(M_TOT, N_IN)
    # wt swizzled for contiguous per-partition DMA: [o-slice, k%128, k//128, o]
    wtv = np.ascontiguousarray(
        sv.T.reshape(NKT, P, NOS, O_TILE).transpose(2, 1, 0, 3)
    )
    # g swizzled to [n-rel (partition), block, k-rel]
    g_swz = np.ascontiguousarray(g_all.transpose(1, 0, 2))
    in_maps = [
        {
            "x": np.ascontiguousarray(x2[c * M:(c + 1) * M]),
            "wt": wtv,
            "g": g_swz,
            "ident": np.eye(P, dtype=np.float32),
        }
        for c in range(N_CORES)
    ]
    res = run_bass_kernel_spmd(
        nc,
        in_maps,
        core_ids=list(range(N_CORES)),
        trace=trace,
        **(trace_kwargs or {}),
    )
    full = np.concatenate([r["out"] for r in res.results], axis=0)
    return full.reshape(B, S, N_OUT), res


def kernel(**inputs) -> np.ndarray:
    out, _ = _run(inputs)
    return out

